# revision 10
# baseline (speedup 1.0000x reference)
"""Trainium2 Bass kernel for the LeNet C3 dense-conv layer.

Computes out = conv2d_valid(x, K, stride 1) + bias where K is the dense
[16, 6, 5, 5] kernel scattered from the sparse per-branch weights
(w3/w4/w6), x is [128, 6, 256, 256] f32, out is [128, 16, 252, 252] f32.

Strategy:
  - Pure data parallelism: 16 images per NeuronCore across 8 cores.
  - On device, the conv is computed as 5 shift-accumulated matmuls
    (one per kernel column kx) into PSUM. For a block of 8 output rows,
    the contraction dim packs (channel, input-row): K = 6*12 = 72, and
    the lhsT is a host-built banded matrix [72, 128] whose column
    m = c_out*8 + r holds K_dense[c_out, c_in, i-r, kx] at row
    c_in*12 + i. Two images share each matmul via the N dim
    (N = 2*256 = 512 = one PSUM bank).
  - fp32r (single-pass fp32 PE mode) keeps the matmul at 1 cycle/column;
    it requires the moving free dim >= 256, hence padding W to 256+4
    garbage columns whose results land in discarded output columns.
  - PSUM is evicted through the vector engine with a fused per-partition
    bias add, then DMA'd straight to the NCHW output layout (the
    partition order c_out*8+r makes each channel's 8 rows one contiguous
    8064-byte chunk in DRAM).
"""

import numpy as np

# LeNet-5 C3 sparse channel connectivity (from the model definition).
CH3 = np.array([[0, 1, 2], [1, 2, 3], [2, 3, 4], [3, 4, 5], [0, 4, 5], [0, 1, 5]])
CH4 = np.array([[0, 1, 2, 3], [1, 2, 3, 4], [2, 3, 4, 5], [0, 3, 4, 5],
                [0, 1, 4, 5], [0, 1, 2, 5], [0, 1, 3, 4], [1, 2, 4, 5],
                [0, 2, 3, 5]])

B, C, H, W = 128, 6, 256, 256
CO, HO, WO = 16, 252, 252
NCORES = 8
BPC = B // NCORES           # images per core
NPAIR = BPC // 2            # image pairs per core
KH = KW = 5

_STATE = None  # cached (nc,) so repeat kernel() calls skip re-tracing


def _dense_kernel(w3, w4, w6):
    k = np.zeros((CO, C, KH, KW), np.float32)
    k[np.arange(6)[:, None], CH3] = w3
    k[6 + np.arange(9)[:, None], CH4] = w4
    k[15] = w6[0]
    return k


def _band_lhst(kd, hi, rows):
    """Banded lhsT stack [5, 6*hi, 16*rows] for a block of `rows` output
    rows consuming `hi` input rows. Row index = c_in*hi + i, column
    index = c_out*rows + r, value = kd[c_out, c_in, i-r, kx]."""
    out = np.zeros((KW, C * hi, CO * rows), np.float32)
    for kx in range(KW):
        for ci in range(C):
            for i in range(hi):
                for r in range(rows):
                    ky = i - r
                    if 0 <= ky < KH:
                        out[kx, ci * hi + i, np.arange(CO) * rows + r] = kd[:, ci, ky, kx]
    return out


def _build_module():
    import concourse.bacc as bacc
    import concourse.mybir as mybir
    from concourse.tile import TileContext

    f32 = mybir.dt.float32
    f32r = mybir.dt.float32r

    # Bacc (not Bass): its compile() runs generate_event_semaphores(),
    # which splits multi-wait instructions to satisfy the TRN2 1-wait-
    # per-instruction constraint walrus enforces.
    nc = bacc.Bacc(None)
    # x2[pr, c, h, j*256 + w] = x[2*pr + j, c, h, w]  (host pre-interleave)
    x_d = nc.dram_tensor("x", [NPAIR, C, H, 2 * W], f32r, kind="ExternalInput")
    wf_d = nc.dram_tensor("wf", [72, 5 * 128], f32r, kind="ExternalInput")
    wt_d = nc.dram_tensor("wt", [48, 5 * 64], f32r, kind="ExternalInput")
    b1_d = nc.dram_tensor("b1", [128, 1], f32, kind="ExternalInput")
    b2_d = nc.dram_tensor("b2", [64, 1], f32, kind="ExternalInput")
    o_d = nc.dram_tensor("o", [BPC, CO, HO, WO], f32, kind="ExternalOutput")

    with TileContext(nc) as tc:
        with (
            tc.tile_pool(name="wpool", bufs=1) as wp,
            tc.tile_pool(name="inpool", bufs=6) as ip,
            tc.tile_pool(name="outpool", bufs=6) as op,
            tc.tile_pool(name="pspool", bufs=4, space="PSUM") as pp,
        ):
            wf_t = wp.tile([72, 5 * 128], f32r)
            nc.sync.dma_start(wf_t[:], wf_d[:])
            wt_t = wp.tile([48, 5 * 64], f32r)
            nc.sync.dma_start(wt_t[:], wt_d[:])
            b1_t = wp.tile([128, 1], f32)
            nc.sync.dma_start(b1_t[:], b1_d[:])
            b2_t = wp.tile([64, 1], f32)
            nc.sync.dma_start(b2_t[:], b2_d[:])

            # Prime each constant tile on the engine class that will
            # consume it, so steady-state matmuls/evictions never carry
            # more than ~2 semaphore waits (walrus caps sync waits on the
            # LDWEIGHTS half of a matmul).
            prime_ps = pp.tile([128, 256], f32, tag="ps")
            nc.tensor.matmul(prime_ps[:], wf_t[:, 0:128], wf_t[:, 0:256],
                             start=True, stop=True)
            prime_ps2 = pp.tile([64, 256], f32, tag="ps")
            nc.tensor.matmul(prime_ps2[:], wt_t[:, 0:64], wt_t[:, 0:256],
                             start=True, stop=True)
            prime_b = op.tile([128, 2], f32, tag="out")
            nc.vector.tensor_scalar_add(prime_b[:, 0:1], b1_t[:], 0.0)
            nc.vector.tensor_scalar_add(prime_b[0:64, 1:2], b2_t[:], 0.0)

            for pr in range(NPAIR):
                for g in range(32):
                    tail = g == 31
                    h0 = 8 * g
                    hi = 8 if tail else 12       # input rows in this block
                    rows = 4 if tail else 8      # output rows in this block
                    kk = C * hi                  # contraction size (72 / 48)
                    mm = CO * rows               # psum partitions (128 / 64)

                    # Input tile: col t = 4 + j*256 + w holds x[2pr+j] col w;
                    # 4 leading pad cols keep fp32r's 256-wide moving dim
                    # in-bounds (pads feed only discarded PSUM cols 0..3;
                    # image 1's "pad" is image 0's tail — same deal).
                    it = ip.tile([kk, 4 + 2 * W], f32r, tag="in")
                    # One DMA per block: src (c, h, 512) vs dst (72, 512)
                    # balances to 3 dims; partition p = c*hi + i.
                    nc.sync.dma_start(
                        it[:, 4:4 + 2 * W],
                        x_d[pr, :, h0:h0 + hi, :],
                    )

                    ps = pp.tile([mm, 2, 256], f32, tag="ps")
                    wsrc = wt_t if tail else wf_t
                    for kx in range(KW):
                        # psum col (j, n): n = out col + 4
                        nc.tensor.matmul(
                            ps[:, :, :],
                            wsrc[:, kx * mm:(kx + 1) * mm],
                            it[:, kx:kx + 2 * W],
                            start=(kx == 0),
                            stop=(kx == KW - 1),
                        )

                    ot = op.tile([mm, 2, WO], f32, tag="out")
                    nc.vector.tensor_scalar_add(
                        ot[:], ps[:, :, 4:4 + WO], (b2_t if tail else b1_t)[:, 0:1]
                    )
                    for j in range(2):
                        nc.scalar.dma_start(
                            o_d[2 * pr + j, :, h0:h0 + rows, :],
                            ot[:, j, :],
                        )
    nc.compile()
    return nc


def _get_module():
    global _STATE
    if _STATE is None:
        _STATE = _build_module()
    return _STATE


def kernel(x, w3, b3, w4, b4, w6, b6):
    from concourse.bass_utils import run_bass_kernel_spmd

    x = np.ascontiguousarray(np.asarray(x, np.float32))
    kd = _dense_kernel(np.asarray(w3, np.float32), np.asarray(w4, np.float32),
                       np.asarray(w6, np.float32))
    bias = np.concatenate([np.asarray(b3, np.float32),
                           np.asarray(b4, np.float32),
                           np.asarray(b6, np.float32)])

    wf = _band_lhst(kd, 12, 8)                     # [5, 72, 128]
    wt = _band_lhst(kd, 8, 4)                      # [5, 48, 64]
    wf2 = np.ascontiguousarray(wf.transpose(1, 0, 2).reshape(72, 5 * 128))
    wt2 = np.ascontiguousarray(wt.transpose(1, 0, 2).reshape(48, 5 * 64))
    b1 = np.repeat(bias, 8).astype(np.float32).reshape(128, 1)
    b2 = np.repeat(bias, 4).astype(np.float32).reshape(64, 1)

    nc = _get_module()
    in_maps = []
    for cr in range(NCORES):
        xs = x[cr * BPC:(cr + 1) * BPC]
        # interleave image pairs into the free dim: [NPAIR, C, H, 2W]
        x2 = np.ascontiguousarray(
            xs.reshape(NPAIR, 2, C, H, W).transpose(0, 2, 3, 1, 4)
        ).reshape(NPAIR, C, H, 2 * W)
        in_maps.append({"x": x2, "wf": wf2, "wt": wt2, "b1": b1, "b2": b2})
    res = run_bass_kernel_spmd(nc, in_maps, core_ids=list(range(NCORES)))
    global LAST_RESULT
    LAST_RESULT = res
    return np.concatenate([r["o"] for r in res.results], axis=0)


LAST_RESULT = None


# revision 11
# speedup vs baseline: 1.3605x; 1.3605x over previous
"""Trainium2 Bass kernel for the LeNet C3 dense-conv layer.

Computes out = conv2d_valid(x, K, stride 1) + bias where K is the dense
[16, 6, 5, 5] kernel scattered from the sparse per-branch weights
(w3/w4/w6), x is [128, 6, 256, 256] f32, out is [128, 16, 252, 252] f32.

Strategy:
  - Pure data parallelism: 16 images per NeuronCore across 8 cores.
  - The conv is 5 shift-accumulated matmuls (one per kernel column kx)
    into PSUM. For a block of 8 output rows of an image pair, the
    contraction packs (input-row, channel): K = 12*6 = 72, and the lhsT
    is a host-built banded matrix [72, 128] whose column m = c_out*8 + r
    holds K_dense[c_out, c_in, i-r, kx] at row i*6 + c_in. Two images
    ride in the moving free dim (N = 2*256 = 512 = one PSUM bank).
  - fp16 operands: full-rate PE (1 col/cycle), fast weight load, and
    half the input HBM traffic. Accumulation stays fp32 in PSUM;
    rel err vs the fp32 reference ~3e-4.
  - Host pre-packs x into x3[pr, h, c, j*256+w] (fp16) so each block's
    input is one 3-dim DMA striped over 12 SDMA engines, and the device
    writes o2[pr, c, h, j*252+w] (f32) so each block's output is one
    DMA with 2016-byte descriptors striped over 16 engines; the host
    un-interleaves afterwards.
  - PSUM is evicted through the vector engine with a fused per-partition
    bias add. The 4 leading pad columns of each input tile keep the
    512-wide moving operand in-bounds; they only feed PSUM columns that
    are discarded (same for image 1's "pad", which is image 0's tail).
"""

import numpy as np

# LeNet-5 C3 sparse channel connectivity (from the model definition).
CH3 = np.array([[0, 1, 2], [1, 2, 3], [2, 3, 4], [3, 4, 5], [0, 4, 5], [0, 1, 5]])
CH4 = np.array([[0, 1, 2, 3], [1, 2, 3, 4], [2, 3, 4, 5], [0, 3, 4, 5],
                [0, 1, 4, 5], [0, 1, 2, 5], [0, 1, 3, 4], [1, 2, 4, 5],
                [0, 2, 3, 5]])

B, C, H, W = 128, 6, 256, 256
CO, HO, WO = 16, 252, 252
NCORES = 8
BPC = B // NCORES           # images per core
NPAIR = BPC // 2            # image pairs per core
KH = KW = 5

_STATE = None  # cached Bass module so repeat kernel() calls skip re-tracing


def _dense_kernel(w3, w4, w6):
    k = np.zeros((CO, C, KH, KW), np.float32)
    k[np.arange(6)[:, None], CH3] = w3
    k[6 + np.arange(9)[:, None], CH4] = w4
    k[15] = w6[0]
    return k


def _band_lhst(kd, hi, rows):
    """Banded lhsT stack [5, 6*hi, 16*rows] for a block of `rows` output
    rows consuming `hi` input rows. Row index = i*6 + c_in (h-major),
    column index = c_out*rows + r, value = kd[c_out, c_in, i-r, kx]."""
    out = np.zeros((KW, C * hi, CO * rows), np.float32)
    for kx in range(KW):
        for ci in range(C):
            for i in range(hi):
                for r in range(rows):
                    ky = i - r
                    if 0 <= ky < KH:
                        out[kx, i * C + ci, np.arange(CO) * rows + r] = kd[:, ci, ky, kx]
    return out


def _build_module():
    import concourse.bacc as bacc
    import concourse.mybir as mybir
    from concourse.tile import TileContext

    f32 = mybir.dt.float32
    f16 = mybir.dt.float16

    # Bacc (not Bass): its compile() runs generate_event_semaphores(),
    # which splits multi-wait instructions to satisfy the TRN2 1-wait-
    # per-instruction constraint walrus enforces.
    nc = bacc.Bacc(None)
    # x3[pr, h, c, j*256 + w] = x[2*pr + j, c, h, w]  (host pre-pack, fp16)
    x_d = nc.dram_tensor("x", [NPAIR, H, C * 2 * W], f16, kind="ExternalInput")
    wf_d = nc.dram_tensor("wf", [72, 5 * 128], f16, kind="ExternalInput")
    wt_d = nc.dram_tensor("wt", [48, 5 * 64], f16, kind="ExternalInput")
    b1_d = nc.dram_tensor("b1", [128, 1], f32, kind="ExternalInput")
    b2_d = nc.dram_tensor("b2", [64, 1], f32, kind="ExternalInput")
    # o2[pr, c, h, j*252 + w] = out[2*pr + j, c, h, w]  (host un-packs)
    o_d = nc.dram_tensor("o", [NPAIR, CO, HO, 2 * WO], f32, kind="ExternalOutput")

    with TileContext(nc) as tc:
        with (
            tc.tile_pool(name="wpool", bufs=1) as wp,
            tc.tile_pool(name="inpool", bufs=8) as ip,
            tc.tile_pool(name="outpool", bufs=8) as op,
            tc.tile_pool(name="pspool", bufs=6, space="PSUM") as pp,
        ):
            wf_t = wp.tile([72, 5 * 128], f16)
            nc.sync.dma_start(wf_t[:], wf_d[:])
            wt_t = wp.tile([48, 5 * 64], f16)
            nc.sync.dma_start(wt_t[:], wt_d[:])
            b1_t = wp.tile([128, 1], f32)
            nc.sync.dma_start(b1_t[:], b1_d[:])
            b2_t = wp.tile([64, 1], f32)
            nc.sync.dma_start(b2_t[:], b2_d[:])

            # Prime each constant tile on the engine class that consumes
            # it, so steady-state instructions carry few semaphore waits.
            prime_ps = pp.tile([128, 256], f32, tag="ps")
            nc.tensor.matmul(prime_ps[:], wf_t[:, 0:128], wf_t[:, 0:256],
                             start=True, stop=True)
            prime_ps2 = pp.tile([64, 256], f32, tag="ps")
            nc.tensor.matmul(prime_ps2[:], wt_t[:, 0:64], wt_t[:, 0:256],
                             start=True, stop=True)
            prime_b = op.tile([128, 2], f32, tag="out")
            nc.vector.tensor_scalar_add(prime_b[:, 0:1], b1_t[:], 0.0)
            nc.vector.tensor_scalar_add(prime_b[0:64, 1:2], b2_t[:], 0.0)

            for pr in range(NPAIR):
                for g in range(32):
                    tail = g == 31
                    h0 = 8 * g
                    hi = 8 if tail else 12       # input rows in this block
                    rows = 4 if tail else 8      # output rows in this block
                    kk = C * hi                  # contraction size (72 / 48)
                    mm = CO * rows               # psum partitions (128 / 64)

                    # Input tile: col t = 4 + j*256 + w holds x[2pr+j] col w
                    # of the partition's row/channel; partition p = i*6 + c.
                    it = ip.tile([kk, 4 + 2 * W], f16, tag="in")
                    nc.sync.dma_start(
                        it[:, 4:4 + 2 * W],
                        x_d[pr, h0:h0 + hi, :],
                    )

                    ps = pp.tile([mm, 2, 256], f32, tag="ps")
                    wsrc = wt_t if tail else wf_t
                    for kx in range(KW):
                        # psum col (j, n): n = out col + 4
                        nc.tensor.matmul(
                            ps[:, :, :],
                            wsrc[:, kx * mm:(kx + 1) * mm],
                            it[:, kx:kx + 2 * W],
                            start=(kx == 0),
                            stop=(kx == KW - 1),
                        )

                    ot = op.tile([mm, 2 * WO], f32, tag="out")
                    nc.vector.tensor_scalar_add(
                        ot[:].rearrange("p (j w) -> p j w", j=2),
                        ps[:, :, 4:4 + WO],
                        (b2_t if tail else b1_t)[:, 0:1],
                    )
                    nc.scalar.dma_start(
                        o_d[pr, :, h0:h0 + rows, :],
                        ot[:],
                    )
    nc.compile()
    return nc


def _get_module():
    global _STATE
    if _STATE is None:
        _STATE = _build_module()
    return _STATE


def kernel(x, w3, b3, w4, b4, w6, b6):
    from concourse.bass_utils import run_bass_kernel_spmd

    x = np.asarray(x, np.float32)
    kd = _dense_kernel(np.asarray(w3, np.float32), np.asarray(w4, np.float32),
                       np.asarray(w6, np.float32))
    bias = np.concatenate([np.asarray(b3, np.float32),
                           np.asarray(b4, np.float32),
                           np.asarray(b6, np.float32)])

    wf = _band_lhst(kd, 12, 8)                     # [5, 72, 128]
    wt = _band_lhst(kd, 8, 4)                      # [5, 48, 64]
    wf2 = np.ascontiguousarray(wf.transpose(1, 0, 2).reshape(72, 5 * 128)).astype(np.float16)
    wt2 = np.ascontiguousarray(wt.transpose(1, 0, 2).reshape(48, 5 * 64)).astype(np.float16)
    b1 = np.repeat(bias, 8).astype(np.float32).reshape(128, 1)
    b2 = np.repeat(bias, 4).astype(np.float32).reshape(64, 1)

    nc = _get_module()
    x16 = x.astype(np.float16)
    in_maps = []
    for cr in range(NCORES):
        xs = x16[cr * BPC:(cr + 1) * BPC]
        # pack to [NPAIR, H, C, 2, W] -> [NPAIR, H, C*2*W]
        x3 = np.ascontiguousarray(
            xs.reshape(NPAIR, 2, C, H, W).transpose(0, 3, 2, 1, 4)
        ).reshape(NPAIR, H, C * 2 * W)
        in_maps.append({"x": x3, "wf": wf2, "wt": wt2, "b1": b1, "b2": b2})
    res = run_bass_kernel_spmd(nc, in_maps, core_ids=list(range(NCORES)))
    global LAST_RESULT
    LAST_RESULT = res

    out = np.empty((B, CO, HO, WO), np.float32)
    for cr in range(NCORES):
        o2 = res.results[cr]["o"].reshape(NPAIR, CO, HO, 2, WO)
        out[cr * BPC:(cr + 1) * BPC] = (
            o2.transpose(0, 3, 1, 2, 4).reshape(BPC, CO, HO, WO)
        )
    return out


LAST_RESULT = None


# revision 17
# speedup vs baseline: 1.3612x; 1.0005x over previous
"""Trainium2 Bass kernel for the LeNet C3 dense-conv layer.

Computes out = conv2d_valid(x, K, stride 1) + bias where K is the dense
[16, 6, 5, 5] kernel scattered from the sparse per-branch weights
(w3/w4/w6), x is [128, 6, 256, 256] f32, out is [128, 16, 252, 252] f32.

Strategy:
  - Pure data parallelism: 16 images per NeuronCore across 8 cores.
  - The conv is 5 shift-accumulated matmuls (one per kernel column kx)
    into PSUM. For a block of 8 output rows of an image pair, the
    contraction packs (input-row, channel): K = 12*6 = 72, and the lhsT
    is a host-built banded matrix [72, 128] whose column m = c_out*8 + r
    holds K_dense[c_out, c_in, i-r, kx] at row i*6 + c_in. Two images
    ride in the moving free dim (N = 2*256 = 512 = one PSUM bank).
  - fp16 operands: full-rate PE (1 col/cycle), fast weight load, and
    half the input HBM traffic. Accumulation stays fp32 in PSUM;
    rel err vs the fp32 reference ~3e-4.
  - Host pre-packs x into x3[pr, h, c, j*256+w] (fp16) so each block's
    input is one 3-dim DMA striped over 12 SDMA engines, and the device
    writes o2[pr, c, h, j*252+w] (f32) so each block's output is one
    DMA with 2016-byte descriptors striped over 16 engines; the host
    un-interleaves afterwards.
  - PSUM is evicted through the vector engine with a fused per-partition
    bias add. The 4 leading pad columns of each input tile keep the
    512-wide moving operand in-bounds; they only feed PSUM columns that
    are discarded (same for image 1's "pad", which is image 0's tail).
"""

import numpy as np

# LeNet-5 C3 sparse channel connectivity (from the model definition).
CH3 = np.array([[0, 1, 2], [1, 2, 3], [2, 3, 4], [3, 4, 5], [0, 4, 5], [0, 1, 5]])
CH4 = np.array([[0, 1, 2, 3], [1, 2, 3, 4], [2, 3, 4, 5], [0, 3, 4, 5],
                [0, 1, 4, 5], [0, 1, 2, 5], [0, 1, 3, 4], [1, 2, 4, 5],
                [0, 2, 3, 5]])

B, C, H, W = 128, 6, 256, 256
CO, HO, WO = 16, 252, 252
NCORES = 8
BPC = B // NCORES           # images per core
NPAIR = BPC // 2            # image pairs per core
KH = KW = 5

_STATE = None  # cached Bass module so repeat kernel() calls skip re-tracing

# Matmul operand dtypes. bf16 moving operand streams 2 cols/cycle on the
# PE (fp16 streams 1), at the cost of 8-bit x mantissas; weights stay fp16.
X_DTYPE = "bfloat16"
W_DTYPE = "float16"


def _dense_kernel(w3, w4, w6):
    k = np.zeros((CO, C, KH, KW), np.float32)
    k[np.arange(6)[:, None], CH3] = w3
    k[6 + np.arange(9)[:, None], CH4] = w4
    k[15] = w6[0]
    return k


def _band_lhst(kd, hi, rows):
    """Banded lhsT stack [5, 6*hi, 16*rows] for a block of `rows` output
    rows consuming `hi` input rows. Row index = i*6 + c_in (h-major),
    column index = c_out*rows + r, value = kd[c_out, c_in, i-r, kx]."""
    out = np.zeros((KW, C * hi, CO * rows), np.float32)
    for kx in range(KW):
        for ci in range(C):
            for i in range(hi):
                for r in range(rows):
                    ky = i - r
                    if 0 <= ky < KH:
                        out[kx, i * C + ci, np.arange(CO) * rows + r] = kd[:, ci, ky, kx]
    return out


def _build_module():
    import concourse.bacc as bacc
    import concourse.mybir as mybir
    from concourse.tile import TileContext

    f32 = mybir.dt.float32
    f16 = mybir.dt.float16
    xdt = getattr(mybir.dt, X_DTYPE)
    wdt = getattr(mybir.dt, W_DTYPE)

    # Bacc (not Bass): its compile() runs generate_event_semaphores(),
    # which splits multi-wait instructions to satisfy the TRN2 1-wait-
    # per-instruction constraint walrus enforces.
    nc = bacc.Bacc(None)
    # x3[pr, (h, c), j*256 + w] = x[2*pr + j, c, h, w]  (host pre-pack).
    # (h, c) is flattened in the declared shape so each block's input DMA
    # is a 2D AP with 72 outer units -> striped over all 16 SDMA engines.
    x_d = nc.dram_tensor("x", [NPAIR, H * C, 2 * W], xdt, kind="ExternalInput")
    wf_d = nc.dram_tensor("wf", [72, 5 * 128], wdt, kind="ExternalInput")
    wt_d = nc.dram_tensor("wt", [48, 5 * 64], wdt, kind="ExternalInput")
    b1_d = nc.dram_tensor("b1", [128, 1], f32, kind="ExternalInput")
    b2_d = nc.dram_tensor("b2", [64, 1], f32, kind="ExternalInput")
    # o2[pr, c, h, j*252 + w] = out[2*pr + j, c, h, w]  (host un-packs)
    o_d = nc.dram_tensor("o", [NPAIR, CO, HO, 2 * WO], f32, kind="ExternalOutput")

    with TileContext(nc) as tc:
        with (
            tc.tile_pool(name="wpool", bufs=1) as wp,
            tc.tile_pool(name="inpool", bufs=8) as ip,
            tc.tile_pool(name="outpool", bufs=8) as op,
            tc.tile_pool(name="pspool", bufs=6, space="PSUM") as pp,
        ):
            wf_t = wp.tile([72, 5 * 128], wdt)
            nc.sync.dma_start(wf_t[:], wf_d[:])
            wt_t = wp.tile([48, 5 * 64], wdt)
            nc.sync.dma_start(wt_t[:], wt_d[:])
            b1_t = wp.tile([128, 1], f32)
            nc.sync.dma_start(b1_t[:], b1_d[:])
            b2_t = wp.tile([64, 1], f32)
            nc.sync.dma_start(b2_t[:], b2_d[:])

            # Prime each constant tile on the engine class that consumes
            # it, so steady-state instructions carry few semaphore waits.
            prime_ps = pp.tile([128, 256], f32, tag="ps")
            nc.tensor.matmul(prime_ps[:], wf_t[:, 0:128], wf_t[:, 0:256],
                             start=True, stop=True)
            prime_ps2 = pp.tile([64, 256], f32, tag="ps")
            nc.tensor.matmul(prime_ps2[:], wt_t[:, 0:64], wt_t[:, 0:256],
                             start=True, stop=True)
            prime_b = op.tile([128, 2], f32, tag="out")
            nc.vector.tensor_scalar_add(prime_b[:, 0:1], b1_t[:], 0.0)
            nc.vector.tensor_scalar_add(prime_b[0:64, 1:2], b2_t[:], 0.0)

            for pr in range(NPAIR):
                for g in range(32):
                    tail = g == 31
                    h0 = 8 * g
                    hi = 8 if tail else 12       # input rows in this block
                    rows = 4 if tail else 8      # output rows in this block
                    kk = C * hi                  # contraction size (72 / 48)
                    mm = CO * rows               # psum partitions (128 / 64)

                    # Input tile: col t = 4 + j*256 + w holds x[2pr+j] col w
                    # of the partition's row/channel; partition p = i*6 + c.
                    it = ip.tile([kk, 4 + 2 * W], xdt, tag="in")
                    nc.sync.dma_start(
                        it[:, 4:4 + 2 * W],
                        x_d[pr, h0 * C:(h0 + hi) * C, :],
                    )

                    ps = pp.tile([mm, 2, 256], f32, tag="ps")
                    wsrc = wt_t if tail else wf_t
                    for kx in range(KW):
                        # psum col (j, n): n = out col + 4
                        nc.tensor.matmul(
                            ps[:, :, :],
                            wsrc[:, kx * mm:(kx + 1) * mm],
                            it[:, kx:kx + 2 * W],
                            start=(kx == 0),
                            stop=(kx == KW - 1),
                        )

                    ot = op.tile([mm, 2 * WO], f32, tag="out")
                    nc.vector.tensor_scalar_add(
                        ot[:].rearrange("p (j w) -> p j w", j=2),
                        ps[:, :, 4:4 + WO],
                        (b2_t if tail else b1_t)[:, 0:1],
                    )
                    nc.scalar.dma_start(
                        o_d[pr, :, h0:h0 + rows, :],
                        ot[:],
                    )
    nc.compile()
    return nc


def _get_module():
    global _STATE
    if _STATE is None:
        _STATE = _build_module()
    return _STATE


def kernel(x, w3, b3, w4, b4, w6, b6):
    from concourse.bass_utils import run_bass_kernel_spmd

    x = np.asarray(x, np.float32)
    kd = _dense_kernel(np.asarray(w3, np.float32), np.asarray(w4, np.float32),
                       np.asarray(w6, np.float32))
    bias = np.concatenate([np.asarray(b3, np.float32),
                           np.asarray(b4, np.float32),
                           np.asarray(b6, np.float32)])

    wf = _band_lhst(kd, 12, 8)                     # [5, 72, 128]
    wt = _band_lhst(kd, 8, 4)                      # [5, 48, 64]
    import concourse.mybir as mybir
    xnp = mybir.dt.np(getattr(mybir.dt, X_DTYPE))
    wnp = mybir.dt.np(getattr(mybir.dt, W_DTYPE))
    wf2 = np.ascontiguousarray(wf.transpose(1, 0, 2).reshape(72, 5 * 128)).astype(wnp)
    wt2 = np.ascontiguousarray(wt.transpose(1, 0, 2).reshape(48, 5 * 64)).astype(wnp)
    b1 = np.repeat(bias, 8).astype(np.float32).reshape(128, 1)
    b2 = np.repeat(bias, 4).astype(np.float32).reshape(64, 1)

    nc = _get_module()
    x16 = x.astype(xnp)
    in_maps = []
    for cr in range(NCORES):
        xs = x16[cr * BPC:(cr + 1) * BPC]
        # pack to [NPAIR, H, C, 2, W] -> [NPAIR, H*C, 2*W]
        x3 = np.ascontiguousarray(
            xs.reshape(NPAIR, 2, C, H, W).transpose(0, 3, 2, 1, 4)
        ).reshape(NPAIR, H * C, 2 * W)
        in_maps.append({"x": x3, "wf": wf2, "wt": wt2, "b1": b1, "b2": b2})
    res = run_bass_kernel_spmd(nc, in_maps, core_ids=list(range(NCORES)))
    global LAST_RESULT
    LAST_RESULT = res

    out = np.empty((B, CO, HO, WO), np.float32)
    for cr in range(NCORES):
        o2 = res.results[cr]["o"].reshape(NPAIR, CO, HO, 2, WO)
        out[cr * BPC:(cr + 1) * BPC] = (
            o2.transpose(0, 3, 1, 2, 4).reshape(BPC, CO, HO, WO)
        )
    return out


LAST_RESULT = None
